# revision 9
# baseline (speedup 1.0000x reference)
"""H2GCN-style GNN message passing on 8 Trainium2 NeuronCores (Bass/Tile).

Sharding: nodes split 1250/core (padded to 1280). Channel-major layouts
throughout so BN/bias are per-partition ops and spmm needs no transposes:
  - hT = relu(x @ feat_W + b)^T as [16ch, 10000], replicated 8x over the
    128 SBUF partitions so all 8 GPSIMD cores gather in parallel.
  - topo: gll_W columns permuted on host so core d's 20480 cols come out
    channel-major; streamed bf16 as the moving matmul operand against the
    stationary flattened gwc vector.
  - spmm (both convs): per-row padded edge slots; ap_gather pulls neighbor
    feature columns, then val-multiply + per-row segment reduce on DVE.
  - c1 AllGather'ed across cores between conv rounds.
  - final linear contracts channel partitions with fin_W split by feature
    group (4 accumulating matmuls), then log_softmax.
"""
import sys
sys.path.insert(0, '/opt/trn_rl_repo')

import numpy as np
import ml_dtypes

import concourse.bass as bass
import concourse.mybir as mybir
import concourse.tile as tile
from concourse import bacc
from concourse import bass_utils

N_CORES = 8
N = 10000
NP = N // N_CORES          # 1250 nodes per core
NPP = 1280                 # padded nodes per core
IN_CH = 256
H = 16
C1 = 32
C2 = 64
NCLASS = 10
KD = 2500                  # len(gwc.flatten())
MP = H * NPP               # 20480 permuted gll_W cols per core
GRP = NPP // 8             # 160 rows per gather group
BN_EPS = 1e-5
SW = NPP                   # topo strip width = one output channel
KCH = (KD + 127) // 128    # 20 k-chunks

f32 = mybir.dt.float32
bf16 = mybir.dt.bfloat16
i16 = mybir.dt.int16

_cache = {}


def _build_program(D1, D2, RP1, RP2):
    nc = bacc.Bacc("TRN2", target_bir_lowering=False, debug=False,
                   enable_asserts=False, num_devices=N_CORES)
    L1, L2 = GRP * D1, GRP * D2

    def din(name, shape, dt):
        return nc.dram_tensor(name, shape, dt, kind="ExternalInput").ap()

    xT = din("xT", [IN_CH, N], f32)
    xTl = din("xTl", [IN_CH, NPP], f32)
    fWr = din("fWr", [IN_CH, 128], f32)
    fWi = din("fWi", [IN_CH, H], f32)
    fbr = din("fbr", [128, 1], f32)
    fb16 = din("fb16", [16, 1], f32)
    gwc = din("gwc", [128, KCH], bf16)
    Wp = din("Wp", [KD, MP], bf16)
    gllb = din("gllb", [H, NPP], f32)
    bnS = din("bnS", [128, 4], f32)
    idx0 = din("idx0", [128, (L1 + L2) // 16], i16)
    val0 = din("val0", [128, L1 + L2], f32)
    idx1 = din("idx1", [128, 2 * (L1 + L2) // 16], i16)
    val1 = din("val1", [128, 2 * (L1 + L2)], f32)
    fwX = din("fwX", [H, NCLASS], f32)
    fwC1 = din("fwC1", [C1, NCLASS], f32)
    fwC2 = din("fwC2", [C2, NCLASS], f32)
    fwT = din("fwT", [H, NCLASS], f32)
    fbn = din("fbn", [128, NCLASS], f32)
    out_d = nc.dram_tensor("out", [NPP, NCLASS], f32, kind="ExternalOutput").ap()
    dbg_h = nc.dram_tensor("dbg_h", [128, 2048], f32, kind="ExternalOutput").ap()
    dbg_xs0 = nc.dram_tensor("dbg_xs0", [H, NPP], f32, kind="ExternalOutput").ap()
    dbg_topo = nc.dram_tensor("dbg_topo", [H, NPP], f32, kind="ExternalOutput").ap()
    dbg_c1 = nc.dram_tensor("dbg_c1", [C1, NPP], f32, kind="ExternalOutput").ap()
    dbg_c2 = nc.dram_tensor("dbg_c2", [C2, NPP], f32, kind="ExternalOutput").ap()

    with tile.TileContext(nc) as tc:
        with tc.tile_pool(name="sb", bufs=1) as sb, \
             tc.tile_pool(name="wstream", bufs=2) as ws, \
             tc.tile_pool(name="gbuf", bufs=2) as gpool, \
             tc.tile_pool(name="vbuf", bufs=2) as vpool, \
             tc.tile_pool(name="ps", bufs=2, space="PSUM") as ps, \
             tc.tile_pool(name="pst", bufs=1, space="PSUM") as pst, \
             tc.tile_pool(name="dram", bufs=1, space="DRAM") as dram, \
             tc.tile_pool(name="dram2", bufs=2, space="DRAM") as dram2:

            # ---------- small constants ----------
            fWr_sb = sb.tile([128, 2, 128], f32, tag="fwr")
            nc.sync.dma_start(fWr_sb[:, 0, :], fWr[0:128, :])
            nc.sync.dma_start(fWr_sb[:, 1, :], fWr[128:256, :])
            fWi_sb = sb.tile([128, 2, H], f32, tag="fwi")
            nc.sync.dma_start(fWi_sb[:, 0, :], fWi[0:128, :])
            nc.sync.dma_start(fWi_sb[:, 1, :], fWi[128:256, :])
            fbr_sb = sb.tile([128, 1], f32, tag="fbr")
            nc.sync.dma_start(fbr_sb[:], fbr[:])
            fb16_sb = sb.tile([16, 1], f32, tag="fb16")
            nc.sync.dma_start(fb16_sb[:], fb16[:])
            gwc_sb = sb.tile([128, KCH], bf16, tag="gwc")
            nc.sync.dma_start(gwc_sb[:], gwc[:])
            bn_sb = sb.tile([128, 4], f32, tag="bn")
            nc.sync.dma_start(bn_sb[:], bnS[:])
            fwX_sb = sb.tile([H, NCLASS], f32, tag="fwX")
            nc.sync.dma_start(fwX_sb[:], fwX[:])
            fwC1_sb = sb.tile([C1, NCLASS], f32, tag="fwC1")
            nc.sync.dma_start(fwC1_sb[:], fwC1[:])
            fwC2_sb = sb.tile([C2, NCLASS], f32, tag="fwC2")
            nc.sync.dma_start(fwC2_sb[:], fwC2[:])
            fwT_sb = sb.tile([H, NCLASS], f32, tag="fwT")
            nc.sync.dma_start(fwT_sb[:], fwT[:])
            fbn_sb = sb.tile([128, NCLASS], f32, tag="fbn")
            nc.sync.dma_start(fbn_sb[:], fbn[:])
            gllb_sb = sb.tile([H, NPP], f32, tag="gllb")
            nc.sync.dma_start(gllb_sb[:], gllb[:])
            idx0_sb = sb.tile([128, (L1 + L2) // 16], i16, tag="idx0")
            nc.sync.dma_start(idx0_sb[:], idx0[:])
            idx1_sb = sb.tile([128, 2 * (L1 + L2) // 16], i16, tag="idx1")
            nc.sync.dma_start(idx1_sb[:], idx1[:])

            # ---------- phase A: hT (8 replicas) and xs0T ----------
            hT = sb.tile([128, N], f32, tag="table")
            CT = 500
            for t in range(N // CT):
                xt = ws.tile([128, 2, CT], f32, tag="xt")
                nc.sync.dma_start(xt[:, 0, :], xT[0:128, t * CT:(t + 1) * CT])
                nc.sync.dma_start(xt[:, 1, :], xT[128:256, t * CT:(t + 1) * CT])
                hp = ps.tile([128, CT], f32, tag="hps")
                for k in range(2):
                    nc.tensor.matmul(hp[:], lhsT=fWr_sb[:, k, :], rhs=xt[:, k, :],
                                     start=(k == 0), stop=(k == 1))
                nc.scalar.activation(hT[:, t * CT:(t + 1) * CT], hp[:],
                                     mybir.ActivationFunctionType.Relu,
                                     bias=fbr_sb[:], scale=1.0)

            xs0T = sb.tile([H, NPP], f32, tag="xs0")
            for t in range(NPP // 256):
                xt = ws.tile([128, 2, 256], f32, tag="xtl")
                nc.sync.dma_start(xt[:, 0, :], xTl[0:128, t * 256:(t + 1) * 256])
                nc.sync.dma_start(xt[:, 1, :], xTl[128:256, t * 256:(t + 1) * 256])
                hp = ps.tile([H, 256], f32, tag="hps")
                for k in range(2):
                    nc.tensor.matmul(hp[:], lhsT=fWi_sb[:, k, :], rhs=xt[:, k, :],
                                     start=(k == 0), stop=(k == 1))
                nc.scalar.activation(xs0T[:, t * 256:(t + 1) * 256], hp[:],
                                     mybir.ActivationFunctionType.Relu,
                                     bias=fb16_sb[:], scale=1.0)

            # ---------- phase B: topo = gwc_flat @ Wp (+ gll_b) ----------
            # one strip per output channel; psum [1, 1280] -> sbuf scratch ->
            # DRAM bounce -> topoT[ch, :]
            topoT = sb.tile([H, NPP], f32, tag="topoT")
            mm_sl = [(0, 512), (512, 1024), (1024, SW)]
            for s in range(MP // SW):
                tp = pst.tile([1, SW], f32, tag="topops")
                for k in range(KCH):
                    kw = min(128, KD - 128 * k)
                    wt = ws.tile([128, SW], bf16, tag="wp")
                    nc.sync.dma_start(wt[0:kw, :],
                                      Wp[128 * k:128 * k + kw, SW * s:SW * (s + 1)])
                    for m0, m1 in mm_sl:
                        nc.tensor.matmul(tp[:, m0:m1],
                                         lhsT=gwc_sb[0:kw, k:k + 1],
                                         rhs=wt[0:kw, m0:m1],
                                         start=(k == 0), stop=(k == KCH - 1))
                trow = ws.tile([1, SW], f32, tag="trow")
                nc.vector.tensor_copy(trow[:], tp[:])
                tdr = dram2.tile([1, SW], f32, tag="tdr")
                nc.sync.dma_start(tdr[:], trow[:])
                nc.sync.dma_start(topoT[s:s + 1, :], tdr[:])
            nc.vector.tensor_tensor(out=topoT[:], in0=topoT[:], in1=gllb_sb[:],
                                    op=mybir.AluOpType.add)

            # ---------- spmm helper ----------
            def spmm_half(table, nelem, idx_sb, val_dram, voff, ioff, D, RP, cdst):
                """Gather+reduce rows for one (matrix, call) into cdst [128, GRP]."""
                r = 0
                while r < GRP:
                    rp = min(RP, GRP - r)
                    g = gpool.tile([128, RP, D], f32, tag="g")
                    gflat = g[:].rearrange("p r d -> p (r d)")
                    nc.gpsimd.ap_gather(
                        gflat[:, 0:rp * D, None],
                        table[:, :, None],
                        idx_sb[:, (ioff + r * D) // 16:(ioff + (r + rp) * D) // 16],
                        channels=128, num_elems=nelem, d=1, num_idxs=rp * D)
                    v = vpool.tile([128, RP, D], f32, tag="v")
                    nc.sync.dma_start(v[:].rearrange("p r d -> p (r d)")[:, 0:rp * D],
                                      val_dram[:, voff + r * D:voff + (r + rp) * D])
                    nc.vector.tensor_tensor(out=g[:, 0:rp, :], in0=g[:, 0:rp, :],
                                            in1=v[:, 0:rp, :],
                                            op=mybir.AluOpType.mult)
                    nc.vector.tensor_reduce(cdst[:, r:r + rp], g[:, 0:rp, :],
                                            axis=mybir.AxisListType.X,
                                            op=mybir.AluOpType.add)
                    r += rp

            # ---------- phase C: conv0 ----------
            c1loc = sb.tile([C1, NPP], f32, tag="c1loc")
            for mi, (D, L, RP, lo) in enumerate(((D1, L1, RP1, 0),
                                                 (D2, L2, RP2, L1))):
                cpre = sb.tile([128, GRP], f32, tag=f"cpre{mi}")
                spmm_half(hT, N, idx0_sb, val0, lo, lo, D, RP, cpre)
                nc.vector.tensor_scalar(cpre[:], cpre[:],
                                        bn_sb[:, 2 * mi:2 * mi + 1],
                                        bn_sb[:, 2 * mi + 1:2 * mi + 2],
                                        mybir.AluOpType.mult, mybir.AluOpType.add)
                for gi in range(8):
                    nc.sync.dma_start(
                        c1loc[H * mi:H * (mi + 1), GRP * gi:GRP * (gi + 1)],
                        cpre[16 * gi:16 * (gi + 1), :])

            # ---------- AllGather c1 ----------
            c1d = dram.tile([C1, NPP], f32)
            nc.gpsimd.dma_start(c1d[:], c1loc[:])
            c1ag = dram.tile([N_CORES * C1, NPP], f32)
            nc.gpsimd.collective_compute(
                "AllGather", mybir.AluOpType.bypass,
                replica_groups=[list(range(N_CORES))],
                ins=[c1d.opt()], outs=[c1ag.opt()])
            c1rep = sb.tile([128, N_CORES * NPP], f32, tag="table")
            for rep in range(4):
                for kk in range(N_CORES):
                    nc.sync.dma_start(
                        c1rep[C1 * rep:C1 * (rep + 1), NPP * kk:NPP * (kk + 1)],
                        c1ag[C1 * kk:C1 * (kk + 1), :])

            # ---------- phase D: conv1 ----------
            c2T = sb.tile([C2, NPP], f32, tag="c2T")
            for mi, (D, L, RP) in enumerate(((D1, L1, RP1), (D2, L2, RP2))):
                for half in range(2):
                    # sections: [a1A | a1B | a2A | a2B]
                    ioff = 2 * L1 * mi + half * L
                    c2pre = sb.tile([128, GRP], f32, tag=f"c2pre{mi}{half}")
                    spmm_half(c1rep, N_CORES * NPP, idx1_sb, val1,
                              ioff, ioff, D, RP, c2pre)
                    for j in range(4):
                        g = 4 * half + j
                        nc.sync.dma_start(
                            c2T[C1 * mi:C1 * (mi + 1), GRP * g:GRP * (g + 1)],
                            c2pre[C1 * j:C1 * (j + 1), :])

            nc.sync.dma_start(dbg_h[:], hT[:, 0:2048])
            nc.sync.dma_start(dbg_xs0[:], xs0T[:])
            nc.sync.dma_start(dbg_topo[:], topoT[:])
            nc.sync.dma_start(dbg_c1[:], c1loc[:])
            nc.sync.dma_start(dbg_c2[:], c2T[:])
            # ---------- phase E: final linear + log_softmax ----------
            for t in range(NPP // 128):
                sl = slice(128 * t, 128 * (t + 1))
                fo = ps.tile([128, NCLASS], f32, tag="fops")
                nc.tensor.matmul(fo[:], lhsT=xs0T[:, sl], rhs=fwX_sb[:],
                                 start=True, stop=False)
                nc.tensor.matmul(fo[:], lhsT=c1loc[:, sl], rhs=fwC1_sb[:],
                                 start=False, stop=False)
                nc.tensor.matmul(fo[:], lhsT=c2T[:, sl], rhs=fwC2_sb[:],
                                 start=False, stop=False)
                nc.tensor.matmul(fo[:], lhsT=topoT[:, sl], rhs=fwT_sb[:],
                                 start=False, stop=True)
                fosb = sb.tile([128, NCLASS], f32, tag="fosb")
                nc.vector.tensor_tensor(out=fosb[:], in0=fo[:], in1=fbn_sb[:],
                                        op=mybir.AluOpType.add)
                mx = sb.tile([128, 1], f32, tag="mx")
                nc.vector.tensor_reduce(mx[:], fosb[:], axis=mybir.AxisListType.X,
                                        op=mybir.AluOpType.max)
                nc.vector.tensor_scalar(fosb[:], fosb[:], mx[:], None,
                                        mybir.AluOpType.subtract)
                ex = sb.tile([128, NCLASS], f32, tag="ex")
                nc.scalar.activation(ex[:], fosb[:],
                                     mybir.ActivationFunctionType.Exp)
                sm = sb.tile([128, 1], f32, tag="sm")
                nc.vector.tensor_reduce(sm[:], ex[:], axis=mybir.AxisListType.X,
                                        op=mybir.AluOpType.add)
                lg = sb.tile([128, 1], f32, tag="lg")
                nc.scalar.activation(lg[:], sm[:], mybir.ActivationFunctionType.Ln)
                nc.vector.tensor_scalar(fosb[:], fosb[:], lg[:], None,
                                        mybir.AluOpType.subtract)
                nc.sync.dma_start(out_d[sl, :], fosb[:])

    nc.compile()
    return nc


def _pad_deg(deg):
    d = max(4, int(deg))
    return (d + 3) & ~3


def _edge_arrays(row, col, val, d, D, remap):
    """Per-row padded slot arrays for core d. Returns (idx [1280, D] int16,
    val [1280, D] f32)."""
    lod = NP * d
    e0 = np.searchsorted(row, lod)
    e1 = np.searchsorted(row, lod + NP)
    r = (row[e0:e1] - lod).astype(np.int64)
    c = col[e0:e1].astype(np.int64)
    v = val[e0:e1].astype(np.float32)
    starts = np.searchsorted(r, np.arange(NP + 1))
    pos = np.arange(len(r)) - starts[r]
    ip = np.zeros((NPP, D), np.int16)
    vp = np.zeros((NPP, D), np.float32)
    if remap:
        cm = (c // NP) * NPP + (c % NP)
    else:
        cm = c
    ip[r, pos] = cm.astype(np.int16)
    vp[r, pos] = v
    return ip, vp


def _wrap16(slots):
    """[L] logical order -> [16, L//16] wrapped (partition p holds slots
    j*16+p at free j)."""
    return np.ascontiguousarray(slots.reshape(-1, 16).T)


def kernel(x, gwc_feat, a1_row, a1_col, a1_val, a2_row, a2_col, a2_val,
           feat_W, feat_b, gll_W, gll_b, bn_gamma, bn_beta, bn_mean, bn_var,
           fin_W, fin_b):
    x = np.asarray(x, np.float32)
    gwc_feat = np.asarray(gwc_feat, np.float32)
    a1_row = np.asarray(a1_row); a1_col = np.asarray(a1_col)
    a1_val = np.asarray(a1_val, np.float32)
    a2_row = np.asarray(a2_row); a2_col = np.asarray(a2_col)
    a2_val = np.asarray(a2_val, np.float32)
    feat_W = np.asarray(feat_W, np.float32); feat_b = np.asarray(feat_b, np.float32)
    gll_W = np.asarray(gll_W, np.float32); gll_b = np.asarray(gll_b, np.float32)
    fin_W = np.asarray(fin_W, np.float32); fin_b = np.asarray(fin_b, np.float32)

    # global max degrees (same program shape for every core)
    def _maxdeg(row):
        _, cnt = np.unique(row, return_counts=True)
        return int(cnt.max())
    D1 = _pad_deg(_maxdeg(a1_row))
    D2 = _pad_deg(_maxdeg(a2_row))
    # RP multiple of 8 keeps every idx-slice byte offset 4-aligned for the
    # GPSIMD gather ucode (int16 idx pairs are read as 4-byte words)
    RP1 = max(8, min(GRP, (4096 // D1) & ~7))
    RP2 = max(8, min(GRP, (4096 // D2) & ~7))
    L1, L2 = GRP * D1, GRP * D2

    key = (D1, D2, RP1, RP2)
    if key not in _cache:
        _cache[key] = _build_program(D1, D2, RP1, RP2)
    nc = _cache[key]

    # ---- shared host-prep ----
    xT_np = np.ascontiguousarray(x.T)
    fWr_np = np.ascontiguousarray(np.tile(feat_W, (1, 8)))
    fbr_np = np.tile(feat_b, 8).reshape(128, 1).astype(np.float32)
    fb16_np = feat_b.reshape(16, 1).astype(np.float32)
    gwc_flat = gwc_feat.reshape(-1)
    gwc_w = np.zeros((128, KCH), np.float32)
    for k in range(KCH):
        kw = min(128, KD - 128 * k)
        gwc_w[0:kw, k] = gwc_flat[128 * k:128 * k + kw]
    gwc_np = gwc_w.astype(ml_dtypes.bfloat16)
    s = (bn_gamma / np.sqrt(bn_var + BN_EPS)).astype(np.float32)
    tt = (bn_beta - bn_mean * s).astype(np.float32)
    bnS_np = np.zeros((128, 4), np.float32)
    pc = np.arange(128) % 16
    bnS_np[:, 0] = s[pc]; bnS_np[:, 1] = tt[pc]
    bnS_np[:, 2] = s[16 + pc]; bnS_np[:, 3] = tt[16 + pc]
    fwX_np = np.ascontiguousarray(fin_W[0:16])
    fwC1_np = np.ascontiguousarray(fin_W[16:48])
    fwC2_np = np.ascontiguousarray(fin_W[48:112])
    fwT_np = np.ascontiguousarray(fin_W[112:128])
    fbn_np = np.tile(fin_b, (128, 1)).astype(np.float32)

    in_maps = []
    for d in range(N_CORES):
        # local x
        xTl_np = np.zeros((IN_CH, NPP), np.float32)
        xTl_np[:, 0:NP] = x[NP * d:NP * (d + 1)].T
        # permuted gll_W shard: col ch*1280+n  <-  gll col 16*(1250 d + n)+ch
        Wd = gll_W[:, H * NP * d:H * NP * (d + 1)].reshape(KD, NP, H)
        Wp_np = np.zeros((KD, H, NPP), ml_dtypes.bfloat16)
        Wp_np[:, :, 0:NP] = Wd.transpose(0, 2, 1).astype(ml_dtypes.bfloat16)
        Wp_np = Wp_np.reshape(KD, MP)
        bd = gll_b[H * NP * d:H * NP * (d + 1)].reshape(NP, H)
        gllb_np = np.zeros((H, NPP), np.float32)
        gllb_np[:, 0:NP] = bd.T

        i1, v1 = _edge_arrays(a1_row, a1_col, a1_val, d, D1, False)
        i2, v2 = _edge_arrays(a2_row, a2_col, a2_val, d, D2, False)
        j1, w1 = _edge_arrays(a1_row, a1_col, a1_val, d, D1, True)
        j2, w2 = _edge_arrays(a2_row, a2_col, a2_val, d, D2, True)

        idx0_np = np.zeros((128, (L1 + L2) // 16), np.int16)
        val0_np = np.zeros((128, L1 + L2), np.float32)
        for g in range(8):
            rs = slice(GRP * g, GRP * (g + 1))
            ps16 = slice(16 * g, 16 * (g + 1))
            idx0_np[ps16, 0:L1 // 16] = _wrap16(i1[rs].reshape(-1))
            idx0_np[ps16, L1 // 16:] = _wrap16(i2[rs].reshape(-1))
            val0_np[ps16, 0:L1] = v1[rs].reshape(1, -1)
            val0_np[ps16, L1:] = v2[rs].reshape(1, -1)

        idx1_np = np.zeros((128, 2 * (L1 + L2) // 16), np.int16)
        val1_np = np.zeros((128, 2 * (L1 + L2)), np.float32)
        for half in range(2):
            for j in range(4):
                g = 4 * half + j
                rs = slice(GRP * g, GRP * (g + 1))
                ps32 = slice(32 * j, 32 * (j + 1))
                o1 = half * L1           # a1 section offset (free)
                o2 = 2 * L1 + half * L2  # a2 section offset
                w16a = _wrap16(j1[rs].reshape(-1))
                w16b = _wrap16(j2[rs].reshape(-1))
                idx1_np[ps32, o1 // 16:(o1 + L1) // 16] = np.tile(w16a, (2, 1))
                idx1_np[ps32, o2 // 16:(o2 + L2) // 16] = np.tile(w16b, (2, 1))
                val1_np[ps32, o1:o1 + L1] = w1[rs].reshape(1, -1)
                val1_np[ps32, o2:o2 + L2] = w2[rs].reshape(1, -1)

        in_maps.append({
            "xT": xT_np, "xTl": xTl_np, "fWr": fWr_np, "fWi": feat_W,
            "fbr": fbr_np, "fb16": fb16_np, "gwc": gwc_np, "Wp": Wp_np,
            "gllb": gllb_np, "bnS": bnS_np,
            "idx0": idx0_np, "val0": val0_np,
            "idx1": idx1_np, "val1": val1_np,
            "fwX": fwX_np, "fwC1": fwC1_np, "fwC2": fwC2_np, "fwT": fwT_np,
            "fbn": fbn_np,
        })

    res = bass_utils.run_bass_kernel_spmd(nc, in_maps,
                                          core_ids=list(range(N_CORES)))
    out = np.concatenate([res.results[d]["out"][0:NP] for d in range(N_CORES)],
                         axis=0)
    return out.astype(np.float32)
